# revision 5
# baseline (speedup 1.0000x reference)
"""Complex GRU cell on 8 Trainium2 NeuronCores (Bass/Tile) — fp8 DoubleRow.

Strategy v2 (over the fp16 baseline):
  - Matmuls in fp8e4 with MatmulPerfMode.DoubleRow (0.5 cycles/row, 2
    k-tiles per instruction).  Precision is recovered with a hi/lo split:
    every matmul operand X is stored as X_hi = e4m3(X) and
    X_lo = e4m3(X - X_hi) at the same scale; each Gauss product computes
    hi*hi + hi*lo + lo*hi (the lo*lo term is ~2^-16 relative and dropped).
    End-to-end rel err ~3e-3 (gate 2e-2).
  - Cross-kt DR pairing: all three split terms pair k-tiles (kt,kt+1)
    with a fixed hi/lo slot, so no broadcast/duplicated operands.
  - Everything on device lives in the x512 scale domain (= S_W*S_A of the
    fp8 scales, an exact power of two): psum results, drained pre-acts
    (with bias*512 folded into the drain's Identity copy), h (host-scaled
    x512 fp16), h_tilde, and the blended output (host divides by 512).
    ACT's scale/bias ports absorb every rescale, so no extra DVE ops.
  - The polar-tanh magnitude chain uses AF.Rsqrt (+eps via the ACT bias
    port) instead of the exponent-bits Exp trick: 1 ACT op replaces 6.
    Table sets alternate Sigmoid <-> Rsqrt once per direction per chunk
    (tanh runs among the sigmoids of the next chunk's R/Z waves).
  - r*h is quantized on device: the (re,im) pair is converted hi on ACT
    (Identity, scale 2^-5) and lo on DVE; the sum plane on DVE/Pool.
  - Data-parallel over 8 cores (batch 16384 -> 8 x 2048), weights
    replicated; same R -> Z -> C wave pipeline as the baseline with
    beta(prev) interleaved into the next sub-chunk's R/Z waves.
"""
import sys

for _p in ("/opt/trn_rl_repo",):
    if _p not in sys.path:
        sys.path.insert(0, _p)

import numpy as np
import ml_dtypes
import concourse.bass as bass
import concourse.tile as tile
import concourse.mybir as mybir
from concourse.bass_utils import run_bass_kernel_spmd

F32, F16, F8 = mybir.dt.float32, mybir.dt.float16, mybir.dt.float8e4
E4 = ml_dtypes.float8_e4m3
AF = mybir.ActivationFunctionType
ALU = mybir.AluOpType
DR = mybir.MatmulPerfMode.DoubleRow

RE, IM, IMN = 0, 1, 2  # weight variant slots: Wr, (Wi-Wr), -(Wr+Wi)
GZ, GR, GH = 0, 1, 2   # gates (z, r, candidate)

N_CORES = 8
B_FULL, D, H = 16384, 512, 512
B_LOCAL = B_FULL // N_CORES
BCHUNK = 512
NBC = B_LOCAL // BCHUNK
SUBS = [(0, 0, 512), (1, 0, 512), (2, 0, 512), (3, 0, 512)]

S_W, S_A = 32.0, 16.0          # fp8 scales (weights, activations)
SCL = S_W * S_A                # 512 = 2^9, the device scale domain
INV_SCL = 1.0 / SCL            # ACT scale to return to unscaled domain
RH_Q = 1.0 / S_W               # rh (x512) -> fp8 plane (x16)
EPS = 6.2e-5

LAST_RUN_INFO = {}
_CACHE = {}


def _split_waits(nc, maxw=1):
    """walrus allows 1 sync wait per instruction; hoist extras onto NoOps."""
    for fn in nc.m.functions:
        for bb in fn.blocks:
            out = []
            for inst in list(bb.instructions):
                si = inst.sync_info
                waits = list(si.on_wait) if si is not None else []
                if len(waits) > maxw:
                    extra, keep = waits[:-maxw], waits[-maxw:]
                    k = 0
                    while extra:
                        chunk, extra = extra[:maxw], extra[maxw:]
                        out.append(mybir.InstNoOp(
                            name=f"{inst.name}-wsplit{k}", engine=inst.engine,
                            ins=[], outs=[],
                            sync_info=mybir.SyncInfo(on_wait=chunk, on_update=[])))
                        k += 1
                    inst.sync_info = mybir.SyncInfo(on_wait=keep,
                                                    on_update=list(si.on_update))
                out.append(inst)
            bb.instructions[:] = out
    return nc


def _build(split_for_hw=True):
    nc = bass.Bass("TRN2", target_bir_lowering=False, debug=False)

    dram_acts = {}
    for nm in ("xr", "xi", "xs", "hr", "hi", "hs"):
        dram_acts[nm] = nc.dram_tensor(nm, [NBC, 128, 4, 2, BCHUNK], F8,
                                       kind="ExternalInput")
    dhp = nc.dram_tensor("hp", [NBC, 128, 4, 2, BCHUNK], F16,
                         kind="ExternalInput")
    wx = nc.dram_tensor("wx", [3, 3, 128, 4, 2, 512], F8, kind="ExternalInput")
    wh = nc.dram_tensor("wh", [2, 3, 128, 4, 2, 512], F8, kind="ExternalInput")
    whc = nc.dram_tensor("whc", [3, 128, 4, 512], F16, kind="ExternalInput")
    # bias slots per (g,t4): 0 = (bxr+bhr)*512, 1 = db = (bxi+bhi)-(bxr+bhr),
    # 2 = (bxi+bhi)*512; slot 36 = EPS for the Sqrt clamp.
    dbias = nc.dram_tensor("bias", [128, 37], F32, kind="ExternalInput")
    outp = nc.dram_tensor("outp", [NBC, 128, 4, 2, BCHUNK], F16,
                          kind="ExternalOutput")

    with tile.TileContext(nc) as tc:
        with (
            tc.tile_pool(name="wpool", bufs=1) as wpool,
            tc.tile_pool(name="apool", bufs=2) as apool,
            tc.tile_pool(name="hpool", bufs=1) as hpool,
            tc.tile_pool(name="rhpool", bufs=1) as rhpool,
            tc.tile_pool(name="zpool", bufs=1) as zpool,
            tc.tile_pool(name="cpool", bufs=1) as cpool,
            tc.tile_pool(name="s1pool", bufs=1) as s1pool,
            tc.tile_pool(name="opool", bufs=2) as opool,
            tc.tile_pool(name="pspool", bufs=8, space="PSUM") as pspool,
        ):
            W = {}

            def load_w(g, v):
                t = wpool.tile([128, 4, 2, 512], F8, tag=f"wx{g}{v}")
                nc.sync.dma_start(t[:], wx[g, v])
                W[("x", g, v)] = t
                if g == GH:
                    t = wpool.tile([128, 4, 512], F16, tag=f"whc{v}")
                    nc.sync.dma_start(t[:], whc[v])
                else:
                    t = wpool.tile([128, 4, 2, 512], F8, tag=f"wh{g}{v}")
                    nc.sync.dma_start(t[:], wh[g, v])
                W[("h", g, v)] = t

            def load_acts(bc, names=("xr", "xi", "xs"), hp=True):
                d = {}
                for nm in names:
                    t = apool.tile([128, 4, 2, BCHUNK], F8, tag=f"a{nm}")
                    nc.sync.dma_start(t[:], dram_acts[nm][bc])
                    d[nm] = t
                if hp:
                    t = apool.tile([128, 4, 2, BCHUNK], F16, tag="ahp")
                    nc.sync.dma_start(t[:], dhp[bc])
                    d["hp"] = t
                return d

            def load_acts_h(bc):
                d = {}
                for nm in ("hr", "hi", "hs"):
                    t = hpool.tile([128, 4, 2, BCHUNK], F8, tag=f"a{nm}")
                    nc.sync.dma_start(t[:], dram_acts[nm][bc])
                    d[nm] = t
                return d

            # ---- startup DMA, ordered by first use ----
            # order matched to PE exhaustion: the scheduler drains all
            # ready IMN/IM groups first, so weights must lead their acts
            act0 = {}

            def _a0(nm, pool):
                t = pool.tile([128, 4, 2, BCHUNK], F8, tag=f"a{nm}")
                nc.sync.dma_start(t[:], dram_acts[nm][0])
                act0[nm] = t

            def _w1(which, g, v):
                if which == "x":
                    t = wpool.tile([128, 4, 2, 512], F8, tag=f"wx{g}{v}")
                    nc.sync.dma_start(t[:], wx[g, v])
                    W[("x", g, v)] = t
                else:
                    t = wpool.tile([128, 4, 2, 512], F8, tag=f"wh{g}{v}")
                    nc.sync.dma_start(t[:], wh[g, v])
                    W[("h", g, v)] = t

            _a0("xi", apool)
            _w1("x", GR, IMN)
            _a0("hi", hpool)
            _w1("h", GR, IMN)
            _a0("xr", apool)
            _w1("x", GR, IM)
            _a0("hr", hpool)
            _w1("h", GR, IM)
            _a0("xs", apool)
            _w1("x", GR, RE)
            _a0("hs", hpool)
            _w1("h", GR, RE)
            load_w(GZ, IMN)
            t = apool.tile([128, 4, 2, BCHUNK], F16, tag="ahp")
            nc.sync.dma_start(t[:], dhp[0])
            act0["hp"] = t
            btile = wpool.tile([128, 37], F32, tag="bias")
            nc.sync.dma_start(btile[:], dbias[:, :])
            for v in (IM, RE):
                load_w(GZ, v)
            for v in (IMN, IM, RE):
                load_w(GH, v)

            def b_ap(g, t4, comp):
                idx = (g * 4 + t4) * 3 + comp
                return btile[:, idx:idx + 1]

            eps_ap = btile[:, 36:37]

            VAR_ACT = {IMN: "xi", IM: "xr", RE: "xs"}
            VAR_H = {IMN: "hi", IM: "hr", RE: "hs"}
            VAR_RH = {IMN: ("p", 1), IM: ("p", 0), RE: ("s", None)}

            def mm_group(ps, g, v, t4, ck, rh=None):
                """One Gauss product group into psum tile ps.  fp8 sides
                are 6 DR matmuls: (hi*hi, lo_w*hi_a, hi_w*lo_a) x kt pairs;
                the candidate's rh side is 4 plain fp16 matmuls (rh is
                produced on device, so fp16 skips the fp8 hi/lo quantize)."""
                act, off, w = ck["act"], ck["off"], ck["W"]
                t4s = slice(t4 * 128, (t4 + 1) * 128)
                cs = slice(off, off + w)
                wt = W[("x", g, v)]
                n = 6 + (4 if rh is not None else 6)
                i = 0
                for kt in (0, 2):
                    ks = slice(kt, kt + 2)
                    for ws, as_ in ((0, 0), (1, 0), (0, 1)):
                        nc.tensor.matmul(
                            ps[:, :w], wt[:, ks, ws, t4s],
                            act[VAR_ACT[v]][:, ks, as_, cs],
                            start=(i == 0), stop=(i == n - 1),
                            perf_mode=DR)
                        i += 1
                if rh is None:
                    wt = W[("h", g, v)]
                    src = act[VAR_H[v]]
                    for kt in (0, 2):
                        ks = slice(kt, kt + 2)
                        for ws, as_ in ((0, 0), (1, 0), (0, 1)):
                            nc.tensor.matmul(
                                ps[:, :w], wt[:, ks, ws, t4s],
                                src[:, ks, as_, cs],
                                start=False, stop=(i == n - 1),
                                perf_mode=DR)
                            i += 1
                else:
                    wt = W[("h", g, v)]  # fp16 [128, 4, 512]
                    kind, tens = VAR_RH[v]
                    for kt in range(4):
                        if kind == "p":
                            sap = rh["p"][:, kt, tens, cs]
                        else:
                            sap = rh["s"][:, kt, cs]
                        nc.tensor.matmul(
                            ps[:, :w], wt[:, kt, t4s], sap,
                            start=False, stop=(i == n - 1))
                        i += 1

            def drain(A, Bk, C, g, t4, w, pool, tag, two_csb=False):
                """P = (pre + bias)*512 as an (re,im) pair tile; csb carries
                the re-bias.  With two_csb the im half gets its own fully
                biased copy (candidate gate); otherwise the im db correction
                rides the consumer's ACT bias port.  High priority: the adds
                release PSUM banks, which gates the PE pipeline."""
                ctx = tc.high_priority(offset=35)
                ctx.__enter__()
                csb = s1pool.tile([128, BCHUNK], F16, tag=f"csb{t4 % 2}")
                nc.scalar.activation(csb[:, :w], C[:, :w], AF.Identity,
                                     bias=b_ap(g, t4, 0))
                P = pool.tile([128, 2, BCHUNK], F16, tag=tag)
                nc.vector.tensor_tensor(P[:, 0, :w], A[:, :w], csb[:, :w],
                                        ALU.add)
                if two_csb:
                    csbi = s1pool.tile([128, BCHUNK], F16,
                                       tag="csbi")
                    nc.scalar.activation(csbi[:, :w], C[:, :w], AF.Identity,
                                         bias=b_ap(g, t4, 2))
                    nc.vector.tensor_tensor(P[:, 1, :w], Bk[:, :w],
                                            csbi[:, :w], ALU.add)
                else:
                    nc.vector.tensor_tensor(P[:, 1, :w], Bk[:, :w],
                                            csb[:, :w], ALU.add)
                ctx.__exit__(None, None, None)
                return P

            def pump(chains, rounds=1, all_=False):
                while chains:
                    for gch in list(chains):
                        try:
                            next(gch)
                        except StopIteration:
                            chains.remove(gch)
                    if not all_:
                        rounds -= 1
                        if rounds <= 0:
                            break

            def wave(g, ck, rh=None, sink_gen=None, pool=None, tag=None,
                     two_csb=False, ptag4=False):
                """Per t4: three 12-DR Gauss groups, drain, then the sink
                chain ISSUED STEP-INTERLEAVED across t4s (engine streams are
                in-order; chain-major issue would serialize the chains)."""
                chains = []
                for t4 in range(4):
                    A = pspool.tile([128, BCHUNK], F32, tag="ps", name="ps")
                    mm_group(A, g, IMN, t4, ck, rh=rh)
                    Bk = pspool.tile([128, BCHUNK], F32, tag="ps", name="ps")
                    mm_group(Bk, g, IM, t4, ck, rh=rh)
                    C = pspool.tile([128, BCHUNK], F32, tag="ps", name="ps")
                    mm_group(C, g, RE, t4, ck, rh=rh)
                    P = drain(A, Bk, C, g, t4, ck["W"], pool,
                              f"{tag}{t4 if ptag4 else t4 % 2}",
                              two_csb=two_csb)
                    chains.append(sink_gen(t4, P))
                    pump(chains, rounds=2)
                pump(chains, all_=True)

            def r_sink(ck, rh_out):
                act, off, w = ck["act"], ck["off"], ck["W"]

                def gen(t4, P):
                    rp = s1pool.tile([128, 2, BCHUNK], F16, tag=f"rp{t4 % 2}")
                    nc.scalar.activation(rp[:, 0, :w], P[:, 0, :w], AF.Sigmoid,
                                         scale=INV_SCL)
                    nc.scalar.activation(rp[:, 1, :w], P[:, 1, :w], AF.Sigmoid,
                                         bias=b_ap(GR, t4, 1), scale=INV_SCL)
                    yield
                    hp = act["hp"]
                    hr4 = hp[:, t4, 0, off:off + w]
                    hi4 = hp[:, t4, 1, off:off + w]
                    t1 = s1pool.tile([128, BCHUNK], F16, tag="t1")
                    t2 = s1pool.tile([128, BCHUNK], F16, tag="t2")
                    nc.vector.tensor_tensor(t1[:, :w], rp[:, 0, :w], hr4,
                                            ALU.mult)
                    nc.vector.tensor_tensor(t2[:, :w], rp[:, 1, :w], hi4,
                                            ALU.mult)
                    t3 = s1pool.tile([128, BCHUNK], F16, tag="t3")
                    t4b = s1pool.tile([128, BCHUNK], F16, tag="t4")
                    nc.gpsimd.tensor_tensor(t3[:, :w], rp[:, 0, :w], hi4,
                                            ALU.mult)
                    nc.gpsimd.tensor_tensor(t4b[:, :w], rp[:, 1, :w], hr4,
                                            ALU.mult)
                    yield
                    # rh pair and sum plane, fp16 x512 (fed straight to the
                    # candidate's fp16 h-side matmuls)
                    rh16, rs16 = rh_out["p"], rh_out["s"]
                    nc.vector.tensor_tensor(rh16[:, t4, 0, :w], t1[:, :w],
                                            t2[:, :w], ALU.subtract)
                    nc.vector.tensor_tensor(rh16[:, t4, 1, :w], t3[:, :w],
                                            t4b[:, :w], ALU.add)
                    yield
                    nc.vector.tensor_tensor(rs16[:, t4, :w],
                                            rh16[:, t4, 0, :w],
                                            rh16[:, t4, 1, :w], ALU.add)
                return gen

            def z_sink(ck, z16):
                w = ck["W"]

                def gen(t4, P):
                    zp = zpool.tile([128, 2, BCHUNK], F16, tag=f"z{t4}")
                    nc.scalar.activation(zp[:, 0, :w], P[:, 0, :w], AF.Sigmoid,
                                         scale=INV_SCL)
                    nc.scalar.activation(zp[:, 1, :w], P[:, 1, :w], AF.Sigmoid,
                                         bias=b_ap(GZ, t4, 1), scale=INV_SCL)
                    z16[t4] = zp
                    yield
                return gen

            def c_sink(ck, cs):
                w = ck["W"]

                def gen(t4, P):
                    # P = (c + b)*512 pair (both components fully biased);
                    # m2 = cr^2+ci^2 unscaled, mag = sqrt(m2+eps), inv=1/mag.
                    cs["P"][t4] = P
                    sre = s1pool.tile([128, BCHUNK], F16, tag="sre")
                    sim_ = s1pool.tile([128, BCHUNK], F16, tag="sim")
                    nc.scalar.activation(sre[:, :w], P[:, 0, :w], AF.Square,
                                         scale=INV_SCL)
                    nc.scalar.activation(sim_[:, :w], P[:, 1, :w], AF.Square,
                                         scale=INV_SCL)
                    yield
                    m2c = s1pool.tile([128, BCHUNK], F16, tag="m2c")
                    nc.vector.tensor_tensor(m2c[:, :w], sre[:, :w],
                                            sim_[:, :w], ALU.add)
                    yield
                    mag = cpool.tile([128, BCHUNK], F16, tag=f"mag{t4}")
                    nc.scalar.activation(mag[:, :w], m2c[:, :w], AF.Sqrt,
                                         bias=eps_ap)
                    cs["mag"][t4] = mag
                    yield
                    inv = cpool.tile([128, BCHUNK], F16, tag=f"inv{t4}")
                    with nc.allow_low_precision(reason="tf=tanh/|c| in fp16"):
                        nc.vector.reciprocal(inv[:, :w], mag[:, :w])
                    cs["inv"][t4] = inv
                    yield
                return gen

            def beta_tanh(ck):
                # tanh in place over mag (mag's only consumer)
                w, th16 = ck["W"], {}
                for t4 in range(4):
                    mag = ck["c"]["mag"][t4]
                    nc.scalar.activation(mag[:, :w], mag[:, :w], AF.Tanh)
                    th16[t4] = mag
                ck["th"] = th16

            beta_ctr = [0]

            def beta_gen(t4, ck, tail=False):
                """h_new = h + z*(h_tilde - h) in the x512 domain."""
                act, off, w = ck["act"], ck["off"], ck["W"]
                par = beta_ctr[0] % 2
                beta_ctr[0] += 1
                P = ck["c"]["P"][t4]
                inv = ck["c"]["inv"][t4]
                zp = ck["z"][t4]
                hp4 = act["hp"][:, t4, :, off:off + w]
                tf = ck["th"][t4]  # tf = tanh*inv in place over th(=mag)
                nc.vector.tensor_tensor(tf[:, :w], tf[:, :w],
                                        inv[:, :w], ALU.mult)
                yield
                ht = s1pool.tile([128, 2, BCHUNK], F16, tag=f"ht{par}")
                nc.vector.tensor_tensor(ht[:, 0, :w], tf[:, :w], P[:, 0, :w],
                                        ALU.mult)
                eng_i = nc.vector if tail else nc.gpsimd
                eng_i.tensor_tensor(ht[:, 1, :w], tf[:, :w], P[:, 1, :w],
                                    ALU.mult)
                yield
                Dp = s1pool.tile([128, 2, BCHUNK], F16, tag=f"Dp{par}")
                nc.vector.tensor_tensor(Dp[:, :, :w], ht[:, :, :w], hp4,
                                        ALU.subtract)
                yield
                U12 = s1pool.tile([128, 2, BCHUNK], F16, tag=f"U12{par}")
                nc.vector.tensor_tensor(U12[:, :, :w], zp[:, :, :w],
                                        Dp[:, :, :w], ALU.mult)
                u3 = s1pool.tile([128, BCHUNK], F16, tag=f"u3{par}")
                u4 = s1pool.tile([128, BCHUNK], F16, tag=f"u4{par}")
                nc.gpsimd.tensor_tensor(u3[:, :w], zp[:, 0, :w], Dp[:, 1, :w],
                                        ALU.mult)
                nc.gpsimd.tensor_tensor(u4[:, :w], zp[:, 1, :w], Dp[:, 0, :w],
                                        ALU.mult)
                yield
                # ere/eim in place into U12's slices
                nc.vector.tensor_tensor(U12[:, 0, :w], U12[:, 0, :w],
                                        U12[:, 1, :w], ALU.subtract)
                nc.vector.tensor_tensor(U12[:, 1, :w], u3[:, :w], u4[:, :w],
                                        ALU.add)
                yield
                O = opool.tile([128, 2, BCHUNK], F16, tag="O")
                nc.vector.tensor_tensor(O[:, :, :w], hp4, U12[:, :, :w],
                                        ALU.add)
                yield
                nc.sync.dma_start(
                    outp[ck["dma"], :, t4, :, off:off + w], O[:, :, :w])

            # ---------------- pipelined main loop --------------------------
            # beta(prev) rides the next chunk's R and Z/C wave sink chains.
            # The LAST chunk runs R -> C -> Z so the heavy candidate sink
            # chains overlap the Z wave's PE time; its tanh+beta ride the
            # z-sink chains (tanh shares the sigmoid table set).
            prev = None
            act = act0
            nsub = len(SUBS)
            for si, (dma, off, w) in enumerate(SUBS):
                ck = {"dma": dma, "off": off, "W": w, "act": act,
                      "c": {"P": {}, "mag": {}, "inv": {}},
                      "z": {}, "th": {}}
                last = si == nsub - 1
                if prev is not None:
                    beta_tanh(prev)

                rhp = rhpool.tile([128, 4, 2, BCHUNK], F16, tag="rhp",
                                  name="rhp")
                rhs_t = rhpool.tile([128, 4, BCHUNK], F16, tag="rhs",
                                    name="rhs")
                rh = {"p": rhp, "s": rhs_t}
                rsink = r_sink(ck, rh)

                def sink_r(t4, P, _rsink=rsink, _prev=prev, _last=last):
                    yield from _rsink(t4, P)
                    if _prev is not None and not _last and t4 in (2, 3):
                        yield from beta_gen(t4 - 2, _prev)

                wave(GR, ck, sink_gen=sink_r, pool=s1pool, tag="Pr")

                ndma = SUBS[si + 1][0] if si + 1 < len(SUBS) else dma
                if ndma != dma:
                    nact = load_acts(ndma)
                else:
                    nact = dict(act)

                if not last:
                    zsink = z_sink(ck, ck["z"])

                    def sink_z(t4, P, _zsink=zsink, _prev=prev):
                        yield from _zsink(t4, P)
                        if _prev is not None and t4 in (0, 2):
                            yield from beta_gen(2 if t4 == 0 else 3, _prev)

                    wave(GZ, ck, sink_gen=sink_z, pool=s1pool, tag="Pz")

                    if ndma != dma:
                        nact.update(load_acts_h(ndma))

                    csink = c_sink(ck, ck["c"])

                    def sink_c(t4, P, _csink=csink):
                        yield from _csink(t4, P)

                    wave(GH, ck, rh=rh, sink_gen=sink_c, pool=cpool,
                         tag="Pc", two_csb=True, ptag4=True)
                else:
                    csink = c_sink(ck, ck["c"])

                    def sink_c(t4, P, _csink=csink, _prev=prev):
                        yield from _csink(t4, P)
                        if _prev is not None:
                            yield from beta_gen(t4, _prev)

                    wave(GH, ck, rh=rh, sink_gen=sink_c, pool=cpool,
                         tag="Pc", two_csb=True, ptag4=True)

                    zsink = z_sink(ck, ck["z"])

                    def sink_z(t4, P, _zsink=zsink, _ck=ck):
                        yield from _zsink(t4, P)
                        mag = _ck["c"]["mag"][t4]
                        nc.scalar.activation(mag[:, :_ck["W"]],
                                             mag[:, :_ck["W"]], AF.Tanh)
                        _ck["th"][t4] = mag
                        yield
                        yield from beta_gen(t4, _ck, tail=True)

                    wave(GZ, ck, sink_gen=sink_z, pool=s1pool, tag="Pz")

                prev = ck
                act = nact

    if split_for_hw:
        _split_waits(nc)
    return nc


def _q8pair(a, scale):
    """[..., n] float32 -> hi/lo e4m3 stacked on a new axis -2."""
    s = (a * scale).astype(np.float32)
    hi = s.astype(E4)
    lo = (s - hi.astype(np.float32)).astype(E4)
    return hi, lo


def _prep(inputs):
    x_re, x_im = inputs["x_re"], inputs["x_im"]
    h_re, h_im = inputs["h_re"], inputs["h_im"]

    def act8(a, sl):
        # [B_LOCAL, 512] -> [NBC, 128, 4kt, 2(hi/lo), BCHUNK] fp8 (x S_A)
        v = a[sl].T.reshape(4, 128, NBC, BCHUNK)  # [kt, p, nbc, col]
        hi, lo = _q8pair(v, S_A)
        out = np.stack([hi, lo], axis=2)          # [kt, p, 2, nbc, col]
        return np.ascontiguousarray(out.transpose(3, 1, 0, 2, 4))

    def wvar8(Wre, Wim, gates):
        out = np.empty((len(gates), 3, 128, 4, 2, 512), E4)
        for gi, g in enumerate(gates):
            WreT, WimT = Wre[g].T, Wim[g].T  # [in, out]
            for v, m in ((RE, WreT), (IM, WimT - WreT), (IMN, -(WreT + WimT))):
                t = m.reshape(4, 128, 512)   # [kt, p, out]
                hi, lo = _q8pair(t, S_W)
                out[gi, v, :, :, 0] = hi.transpose(1, 0, 2)
                out[gi, v, :, :, 1] = lo.transpose(1, 0, 2)
        return out

    wxn = wvar8(inputs["Wx_re"], inputs["Wx_im"], (0, 1, 2))
    whn = wvar8(inputs["Wh_re"], inputs["Wh_im"], (0, 1))
    # candidate h-side weights, fp16 unscaled: [3v, 128, 4kt, 512]
    WreT, WimT = inputs["Wh_re"][2].T, inputs["Wh_im"][2].T
    whcn = np.empty((3, 128, 4, 512), np.float16)
    for v, m in ((RE, WreT), (IM, WimT - WreT), (IMN, -(WreT + WimT))):
        whcn[v] = m.reshape(4, 128, 512).transpose(1, 0, 2)

    def hpair(sl):
        # [NBC, 128, 4t4, 2(re/im), BCHUNK] fp16, x512
        vr = (h_re[sl].T.reshape(4, 128, NBC, BCHUNK) * SCL)
        vi = (h_im[sl].T.reshape(4, 128, NBC, BCHUNK) * SCL)
        v = np.stack([vr, vi], axis=2)  # [t4, p, 2, nbc, col]
        return np.ascontiguousarray(
            v.transpose(3, 1, 0, 2, 4)).astype(np.float16)

    # bias table
    br = inputs["bx_re"] + inputs["bh_re"]   # [3, 512]
    bi = inputs["bx_im"] + inputs["bh_im"]
    bias = np.zeros((128, 37), np.float32)
    for g in range(3):
        for t4 in range(4):
            seg_r = br[g, t4 * 128:(t4 + 1) * 128]
            seg_i = bi[g, t4 * 128:(t4 + 1) * 128]
            base = (g * 4 + t4) * 3
            bias[:, base + 0] = seg_r * SCL
            bias[:, base + 1] = seg_i - seg_r
            bias[:, base + 2] = seg_i * SCL
    bias[:, 36] = EPS

    in_maps = []
    for c in range(N_CORES):
        sl = slice(c * B_LOCAL, (c + 1) * B_LOCAL)
        in_maps.append({
            "xr": act8(x_re, sl), "xi": act8(x_im, sl),
            "xs": act8(np.asarray(x_re, np.float16) +
                       np.asarray(x_im, np.float16), sl),
            "hr": act8(h_re, sl), "hi": act8(h_im, sl),
            "hs": act8(np.asarray(h_re, np.float16) +
                       np.asarray(h_im, np.float16), sl),
            "hp": hpair(sl),
            "wx": wxn, "wh": whn, "whc": whcn, "bias": bias,
        })
    return in_maps


def kernel(**inputs):
    if "nc" not in _CACHE:
        nc = _build(split_for_hw=False)
        try:
            from concourse.timeline_sim import TimelineSim
            LAST_RUN_INFO["timeline_ns"] = int(TimelineSim(nc).simulate())
        except Exception:
            pass
        _CACHE["nc"] = _split_waits(nc)
    nc = _CACHE["nc"]

    in_maps = _prep(inputs)
    res = run_bass_kernel_spmd(nc, in_maps, list(range(N_CORES)))
    LAST_RUN_INFO["exec_time_ns"] = res.exec_time_ns

    out = np.empty((B_FULL, 512, 2), np.float32)
    for c, r in enumerate(res.results):
        sl = slice(c * B_LOCAL, (c + 1) * B_LOCAL)
        # outp [NBC, 128p, 4t4, 2, col] -> [B, 512, 2]
        o = r["outp"].astype(np.float32) * (1.0 / SCL)
        out[sl] = o.transpose(0, 4, 2, 1, 3).reshape(B_LOCAL, 512, 2)
    return out
